# revision 47
# baseline (speedup 1.0000x reference)
"""Trainium2 Bass kernel for nn_Attention: per-head QKV attention + out-proj.

Contract: kernel(**inputs) takes FULL unsharded inputs
  x [8, 1024, 768] f32, Wqkv [12, 768, 192] f32, bqkv [12, 192] f32,
  Wo [768, 768] f32, bo [768] f32
returns FULL output [8, 1024, 768] f32.

Strategy: pure data-parallel over batch (8 batches -> 8 NeuronCores), no
collectives.  Each core computes its batch end-to-end in bf16 matmuls.

Math notes:
  - softmax rows sum to 1 => attn @ (v + bv) = attn @ v + bv, and since the
    attention output is immediately projected, bv folds into the projection
    bias: bo2 = bo + concat(bv) @ Wo.  V-bias never touches the device.
  - x is transposed + bf16-cast on HOST (xT [768, 1024]) -- kills the 48 PE
    transposes, the f32 x DMA, and the startup serialization.
  - softmax is computed unnormalized; the denominator r[q] = sum_k et[k, q]
    is built by accumulating the bf16 exp chunks on DVE (S_h += et_chunk,
    flat contiguous [128,1024] ops) and a pair of CONCURRENT col-tiled
    ones-matmuls on the PE (~0.5us) that also broadcast r_a to psum
    partitions 0:64 and r_b to 64:128 (no DRAM bounce, no PE ones-column
    in the PV stationary).  This frees the PV stationary to hold exactly 64
    v-columns per head, so the two heads of a pair also run as CONCURRENT
    col-tiled matmuls (array cols 0-63 / 64-127).
  - output is written bf16 and upcast on host (halves the out DMA).

Schedule: flat software pipeline over 48 (pair, sk) chunks; pv(j-2) rides 2
chunk-slots behind scores(j); v-projection chunks fill pair 0; q/k
projections of pair t+1 injected mid-pair.  All PE instructions chained
with no-sync ordering edges so the Tile scheduler preserves the interleave.
"""

import math
import os

import numpy as np
import ml_dtypes

import concourse.bass as bass
import concourse.tile as tile
from concourse import bacc, mybir
from concourse.bass_utils import run_bass_kernel_spmd
from concourse.tile_rust import add_dep_helper

B, S, D, H, HD = 8, 1024, 768, 12, 64
SCALE = 1.0 / math.sqrt(D)
FP = mybir.dt.float32
BF = mybir.dt.bfloat16
KC = D // 128   # 6 contraction chunks
SC = S // 128   # 8 seq chunks
NQ = S // 512   # 2 free-dim chunks of 512
NP = H // 2     # 6 head pairs

AluOp = mybir.AluOpType
ActFn = mybir.ActivationFunctionType

# Results of the last hardware run (for test harness introspection).
last_results = None


def _build_kernel_body(tc, out_d, xT_d, wqk01_d, wqkr_d, wv_d, wo_d,
                       bqk_d, bo2_d):
    nc = tc.nc

    # Chain every TensorE instruction to the previous one with a no-sync
    # ordering edge: the Tile scheduler otherwise reorders the PE stream by
    # modeled readiness, undoing the deliberate scores/PV/QKV interleave.
    _pe_last = [None]

    def MM(*a, reuse_w=False, **k):
        inst = nc.tensor.matmul(*a, **k)
        if reuse_w:
            # stationary operand identical to the previous matmul in the
            # chained PE stream: skip the redundant LDWEIGHTS (bf16-safe)
            inst.ins.ldweights = False
        if _pe_last[0] is not None:
            add_dep_helper(inst.ins, _pe_last[0].ins, sync=False,
                           reason="pe-order")
        _pe_last[0] = inst
        return inst

    from contextlib import ExitStack

    with ExitStack() as ctx:
        wpool = ctx.enter_context(tc.tile_pool(name="weights", bufs=1))
        bigs = ctx.enter_context(tc.tile_pool(name="bigs", bufs=1))
        etp = ctx.enter_context(tc.tile_pool(name="et", bufs=2))
        spool = ctx.enter_context(tc.tile_pool(name="ssum", bufs=2))
        upool = ctx.enter_context(tc.tile_pool(name="usum", bufs=2))
        rpool = ctx.enter_context(tc.tile_pool(name="rbc", bufs=1))
        rcpool = ctx.enter_context(tc.tile_pool(name="rcp", bufs=1))
        outp = ctx.enter_context(tc.tile_pool(name="outstage", bufs=2))
        # psum budget (8 banks of [128 x 2KB]):
        #   scores transients 2 x 2 banks -- double-buffered so exp(j-1) never
        #     gates scores(j); outproj reuses these in the tail
        #   proj accumulator 1 x 2 banks -- qkv/v/r, so a 2.5us projection
        #     never holds a scores slot hostage (ScalarE starvation)
        #   pv accumulator 1 x 2 banks
        psq = ctx.enter_context(tc.tile_pool(name="ps_t", bufs=2, space="PSUM"))
        pspj = ctx.enter_context(tc.tile_pool(name="ps_pj", bufs=1, space="PSUM"))
        pspv = ctx.enter_context(tc.tile_pool(name="ps_pv", bufs=1, space="PSUM"))

        # ---- persistent sbuf tensors ----
        wqk01_sb = wpool.tile([128, KC, 256], BF)   # q/k blocks m=0 and m=6
        wqkr_sb = wpool.tile([128, KC, 2 * D - 256], BF)  # remaining m blocks
        wv_sb = wpool.tile([128, KC, D], BF)
        wo_sb = wpool.tile([128, KC, D], BF)
        bqk_sb = wpool.tile([128, 2 * KC], FP)
        bo_sb = wpool.tile([128, D], FP)
        oacc = wpool.tile([128, SC, D], BF)         # outproj kc0-4 partials
        xT = bigs.tile([128, KC, S], BF)
        ones_sb = wpool.tile([128, 128], BF)
        nc.vector.memset(ones_sb[:], 1.0)
        # qkT[:, m, :]: m 0..5 -> qT (heads 2m, 2m+1 on partitions 0:64,
        # 64:128), m 6..11 -> kT likewise.
        qkT = bigs.tile([128, 2 * KC, S], BF)
        vsb = bigs.tile([128, SC, D], BF)       # v in [s-part, sk, h*hd]
        outT = bigs.tile([128, KC, S], BF)

        # ---- input DMAs ----
        # All big tensors are prearranged on HOST into partition-contiguous
        # [128, n] layouts matching the sbuf tiles exactly: one descriptor
        # per partition (128 big descs instead of ~768 small) so the rings
        # drain fast.  wqk01 (q/k blocks of heads 0,1) rides first -- it
        # gates pair 0's scores.
        # xT rides the SYNC (SP) queue -- its DGE path starts ~3us earlier
        # than the scalar queue's, and xT gates the very first projection.
        nc.sync.dma_start(xT[:, 0:3, :], xT_d[:, 0:3 * S])
        nc.sync.dma_start(xT[:, 3:KC, :], xT_d[:, 3 * S:])
        nc.scalar.dma_start(wqk01_sb[:], wqk01_d[:, :])
        nc.scalar.dma_start(bqk_sb[:], bqk_d.rearrange("(j p) -> p j", p=128))
        nc.scalar.dma_start(wv_sb[:], wv_d[:, :])
        nc.scalar.dma_start(wqkr_sb[:], wqkr_d[:, :])
        nc.scalar.dma_start(wo_sb[:], wo_d[:, :])
        nc.scalar.dma_start(
            bo_sb[:],
            bo2_d.rearrange("(a f) -> a f", a=1).partition_broadcast(128),
        )

        def wqk_block(m, kc):
            """stationary [128, 128] slice for q/k projection block m"""
            if m == 0:
                return wqk01_sb[:, kc, 0:128]
            if m == KC:
                return wqk01_sb[:, kc, 128:256]
            if m < KC:
                return wqkr_sb[:, kc, (m - 1) * 128:m * 128]
            return wqkr_sb[:, kc, 640 + (m - KC - 1) * 128:640 + (m - KC) * 128]

        def qkv_m(m):
            """project one 128-col block of q or k (m 0..5 q, 6..11 k)"""
            ps = pspj.tile([128, S], FP, tag="pj", name=f"qk_{m}")
            for kc in range(KC):
                lhsT = wqk_block(m, kc)
                for n in range(NQ):
                    MM(
                        ps[:, n * 512:(n + 1) * 512],
                        lhsT,
                        xT[:, kc, n * 512:(n + 1) * 512],
                        start=(kc == 0),
                        stop=(kc == KC - 1),
                        reuse_w=(n > 0),
                    )
            nc.vector.tensor_scalar_add(qkT[:, m, :], ps[:], bqk_sb[:, m:m + 1])

        def v_chunk(sc):
            ps = pspj.tile([128, S], FP, tag="pj", name=f"v_{sc}")
            for kc in range(KC):
                lhsT = xT[:, kc, sc * 128:(sc + 1) * 128]
                MM(ps[:, 0:512], lhsT, wv_sb[:, kc, 0:512],
                   start=(kc == 0), stop=(kc == KC - 1))
                MM(ps[:, 512:D], lhsT, wv_sb[:, kc, 512:D],
                   start=(kc == 0), stop=(kc == KC - 1), reuse_w=True)
            nc.vector.tensor_copy(vsb[:, sc, :], ps[:, 0:D])

        def scores_chunk(t, sk, et_t, s_t):
            for h01 in range(2):
                ps = psq.tile([128, S], FP, tag="ps", name=f"sc_{t}_{sk}_{h01}")
                lo, hi = h01 * 64, (h01 + 1) * 64
                lhsT = qkT[lo:hi, KC + t, sk * 128:(sk + 1) * 128]
                for n in range(NQ):
                    MM(
                        ps[:, n * 512:(n + 1) * 512],
                        lhsT,
                        qkT[lo:hi, t, n * 512:(n + 1) * 512],
                        start=True,
                        stop=True,
                        tile_position=(h01 * 64, 0),
                        reuse_w=(n > 0),
                    )
                nc.scalar.activation(
                    et_t[:, h01, sk, :], ps[:], ActFn.Exp, scale=SCALE
                )
                # running denominator: S_h += et chunk (flat contiguous
                # [128,1024] bf16 ops); first add at sk==1 consumes chunks
                # 0+1, skipping a separate init copy.  Head b's adds for
                # sk<=5 ride the otherwise-idle GpSimd so the DVE queue
                # stays shallow ahead of the pair-boundary u-copy.
                eng = nc.gpsimd if (h01 == 1 and sk <= 5) else nc.vector
                if sk == 1:
                    eng.tensor_tensor(s_t[h01][:], et_t[:, h01, 0, :],
                                      et_t[:, h01, 1, :], op=AluOp.add)
                elif sk > 1:
                    eng.tensor_tensor(s_t[h01][:], s_t[h01][:],
                                      et_t[:, h01, sk, :], op=AluOp.add)

        def pv_chunk(t, sk, et_t, pv_ps):
            # two heads as concurrent col-tiled matmuls: head a -> array
            # cols / psum partitions 0:64, head b -> 64:128 (tile_position
            # auto-derives from the psum slice base partition)
            for n in range(NQ):
                for h01 in range(2):
                    h = 2 * t + h01
                    MM(
                        pv_ps[h01 * 64:(h01 + 1) * 64, n * 512:(n + 1) * 512],
                        vsb[:, sk, h * 64:(h + 1) * 64],
                        et_t[:, h01, sk, n * 512:(n + 1) * 512],
                        start=(sk == 0),
                        stop=(sk == SC - 1),
                        skip_group_check=True,
                    )

        def pv_finalize(t, s_t, pv_ps):
            # Evacuate u from the PV psum IMMEDIATELY (single bf16 copy) so
            # the accumulator frees for the next pair (the strict in-order PE
            # queue would otherwise stall ~5us behind the next pair's first
            # PV matmul).  Then: r_a broadcast to psum partitions 0:64 and
            # r_b to 64:128 via two CONCURRENT col-tiled ones-matmuls
            # (partition-sum of S_h), copy to SBUF, fast Newton reciprocal,
            # and divide u*(1/r) into outT -- all off the PE critical path.
            u = upool.tile([128, S], BF, tag="u", name=f"u_{t}")
            nc.vector.tensor_copy(u[:], pv_ps[:])
            ps_r = pspj.tile([128, S], FP, tag="pj", name=f"r_{t}")
            for h01 in range(2):
                lo, hi = h01 * 64, (h01 + 1) * 64
                for n in range(NQ):
                    MM(ps_r[lo:hi, n * 512:(n + 1) * 512],
                       ones_sb[:, lo:hi],
                       s_t[h01][:, n * 512:(n + 1) * 512],
                       start=True, stop=True,
                       skip_group_check=True)
            rbc = rpool.tile([128, S], FP, tag="rbc", name=f"rbc_{t}")
            nc.vector.tensor_copy(rbc[:], ps_r[:])
            rcp = rcpool.tile([128, S], FP, tag="rcp", name=f"rcp_{t}")
            nc.vector.reciprocal_approx_fast(rcp[:], rbc[:])
            for h01 in range(2):
                lo, hi = h01 * 64, (h01 + 1) * 64
                # head b's divide is off the critical path for pairs 0-4
                # (outT only feeds the out-projection) -> idle GpSimd
                eng = nc.gpsimd if (h01 == 1 and t < NP - 1) else nc.vector
                eng.tensor_tensor(
                    outT[lo:hi, t, :],
                    u[lo:hi, :],
                    rcp[lo:hi, :],
                    op=AluOp.mult,
                )

        # ---- main pipeline ----
        # Flat software pipeline over 48 (pair, sk) chunks: pv(j-2) rides 2
        # chunk-slots behind scores(j), crossing pair boundaries, so neither
        # TensorE nor ScalarE ever drains.
        et_tiles = {}
        s_tiles = {}
        pv_tiles = {}

        def emit_pv(j):
            t, sk = j // SC, j % SC
            if sk == 0:
                pv_tiles[t] = pspv.tile([128, S], FP, tag="pv",
                                        name=f"pv_{t}")
            pv_chunk(t, sk, et_tiles[t], pv_tiles[t])
            if sk == SC - 1:
                pv_finalize(t, s_tiles[t], pv_tiles[t])
                del pv_tiles[t], et_tiles[t], s_tiles[t]

        def outproj_a(sc, pool, tag):
            """accumulate kc 0..4 of the output projection + bo into oacc"""
            ps = pool.tile([128, S], FP, tag=tag, name=f"oa_{sc}")
            for kc in range(KC - 1):
                lhsT = outT[:, kc, sc * 128:(sc + 1) * 128]
                MM(ps[:, 0:512], lhsT, wo_sb[:, kc, 0:512],
                   start=(kc == 0), stop=(kc == KC - 2))
                MM(ps[:, 512:D], lhsT, wo_sb[:, kc, 512:D],
                   start=(kc == 0), stop=(kc == KC - 2), reuse_w=True)
            nc.vector.tensor_tensor(oacc[:, sc, :], ps[:, 0:D], bo_sb[:],
                                    op=AluOp.add)

        # pair 0's q/k projections gate the whole pipeline
        qkv_m(0)
        qkv_m(KC)

        NCH = NP * SC
        for j in range(NCH):
            t, sk = j // SC, j % SC
            if sk == 0:
                et_tiles[t] = etp.tile([128, 2, SC, S], BF, tag="et",
                                       name=f"et_{t}")
                s_tiles[t] = [spool.tile([128, S], BF, tag=f"s{i}",
                                         name=f"s_{t}_{i}") for i in range(2)]
            scores_chunk(t, sk, et_tiles[t], s_tiles[t])
            if t == 0:
                v_chunk(sk)
            # q/k projections of the next pair ride at sk 2/3: the ~2.5us of
            # PE work right after pv(t,7) also absorbs the u-copy latency
            # that gates pv(t+1,0)'s psum slot.
            if t + 1 < NP:
                if sk == 2:
                    qkv_m(t + 1)
                elif sk == 3:
                    qkv_m(KC + t + 1)
            # out-projection kc0-4 partials overlap pair 5 (proj slot is
            # otherwise idle there: no more qkv, no more v); sc=0 waits
            # until sk=3 so finalize(4)'s mult has landed.
            if t == NP - 1 and sk >= 3:
                outproj_a(sk - 3, pspj, "pj")
            if j >= 2:
                emit_pv(j - 2)
        emit_pv(NCH - 2)
        emit_pv(NCH - 1)
        outproj_a(5, pspj, "pj")
        outproj_a(6, psq, "ps")
        outproj_a(7, psq, "ps")

        # ---- output projection: only the kc5 (pair 5) contraction remains ----
        for sc in range(SC):
            ps = psq.tile([128, S], FP, tag="ps", name=f"o_{sc}")
            lhsT = outT[:, KC - 1, sc * 128:(sc + 1) * 128]
            MM(ps[:, 0:512], lhsT, wo_sb[:, KC - 1, 0:512],
               start=True, stop=True)
            MM(ps[:, 512:D], lhsT, wo_sb[:, KC - 1, 512:D],
               start=True, stop=True, reuse_w=True)
            osb = outp.tile([128, D], BF, tag="osb", name=f"osb_{sc}")
            # (GPSIMD cannot access PSUM -- this add must stay on DVE)
            nc.vector.tensor_tensor(osb[:], ps[:, 0:D], oacc[:, sc, :],
                                    op=AluOp.add)
            (nc.scalar if sc % 2 else nc.sync).dma_start(
                out_d[sc * 128:(sc + 1) * 128, :], osb[:])


def build():
    """Build + compile the per-core Bass module. Returns the Bacc object.

    All big inputs are HOST-prearranged into partition-contiguous [128, n]
    layouts that mirror the sbuf tiles (row p = everything partition p
    holds, kc-major), so each DMA is 128 single-span descriptors.
    """
    nc = bacc.Bacc("TRN2", target_bir_lowering=False, debug=False, num_devices=B)
    xT_d = nc.dram_tensor("xT", [128, KC * S], BF, kind="ExternalInput").ap()
    wqk01_d = nc.dram_tensor("wqk01", [128, KC * 256], BF,
                             kind="ExternalInput").ap()
    wqkr_d = nc.dram_tensor("wqkr", [128, KC * (2 * D - 256)], BF,
                            kind="ExternalInput").ap()
    wv_d = nc.dram_tensor("wv", [128, KC * D], BF, kind="ExternalInput").ap()
    wo_d = nc.dram_tensor("wo", [128, KC * D], BF, kind="ExternalInput").ap()
    bqk_d = nc.dram_tensor("bqk", [2 * D], FP, kind="ExternalInput").ap()
    bo2_d = nc.dram_tensor("bo2", [D], FP, kind="ExternalInput").ap()
    out_d = nc.dram_tensor("out", [S, D], BF, kind="ExternalOutput").ap()
    with tile.TileContext(nc) as tc:
        _build_kernel_body(tc, out_d, xT_d, wqk01_d, wqkr_d, wv_d, wo_d,
                           bqk_d, bo2_d)
    nc.compile()
    return nc


def _prearrange(w):
    """[D, F] weight -> partition-contiguous [128, KC*F] (row p holds the
    kc-major sequence of rows kc*128+p), matching tile([128, KC, F])."""
    F = w.shape[1]
    return np.ascontiguousarray(
        w.reshape(KC, 128, F).transpose(1, 0, 2).reshape(128, KC * F))


def prep_weights(Wqkv, bqkv, Wo, bo):
    """Host-side weight packing (numpy only)."""
    # Wqkv [H, D, 3*HD] -> Wq_all/Wk_all/Wv_all [D, H*HD]
    Wq = np.transpose(Wqkv[:, :, 0:HD], (1, 0, 2)).reshape(D, D)
    Wk = np.transpose(Wqkv[:, :, HD:2 * HD], (1, 0, 2)).reshape(D, D)
    Wv = np.transpose(Wqkv[:, :, 2 * HD:], (1, 0, 2)).reshape(D, D)
    bq = bqkv[:, 0:HD].reshape(D)
    bk = bqkv[:, HD:2 * HD].reshape(D)
    bv = bqkv[:, 2 * HD:].reshape(D)
    bqk = np.concatenate([bq, bk])  # [2D]
    bo2 = bo.astype(np.float64) + bv.astype(np.float64) @ Wo.astype(np.float64)
    bf16 = ml_dtypes.bfloat16
    wqk01 = np.concatenate([Wq[:, 0:128], Wk[:, 0:128]], axis=1)  # [D, 256]
    wqkr = np.concatenate([Wq[:, 128:D], Wk[:, 128:D]], axis=1)  # [D, 1280]
    return {
        "wqk01": _prearrange(wqk01.astype(bf16)),
        "wqkr": _prearrange(wqkr.astype(bf16)),
        "wv": _prearrange(Wv.astype(bf16)),
        "wo": _prearrange(Wo.astype(bf16)),
        "bqk": np.ascontiguousarray(bqk.astype(np.float32)),
        "bo2": np.ascontiguousarray(bo2.astype(np.float32)),
    }


def prep_core_inputs(x, Wqkv, bqkv, Wo, bo):
    """Full host-side preprocessing -> list of per-core input maps."""
    w = prep_weights(np.asarray(Wqkv), np.asarray(bqkv), np.asarray(Wo),
                     np.asarray(bo))
    x = np.asarray(x, dtype=np.float32)
    bf16 = ml_dtypes.bfloat16
    return [
        {"xT": _prearrange(np.ascontiguousarray(x[i].T).astype(bf16)), **w}
        for i in range(B)
    ]


_nc_cache = None


def kernel(x, Wqkv, bqkv, Wo, bo):
    global _nc_cache, last_results
    if _nc_cache is None:
        _nc_cache = build()
    nc = _nc_cache
    in_maps = prep_core_inputs(x, Wqkv, bqkv, Wo, bo)
    res = run_bass_kernel_spmd(
        nc, in_maps, core_ids=list(range(B)),
        trace=bool(os.environ.get("KERNEL_TRACE")),
    )
    last_results = res
    out = np.stack([res.results[i]["out"] for i in range(B)], axis=0)
    return out.astype(np.float32)


# revision 48
# speedup vs baseline: 1.0084x; 1.0084x over previous
"""Trainium2 Bass kernel for nn_Attention: per-head QKV attention + out-proj.

Contract: kernel(**inputs) takes FULL unsharded inputs
  x [8, 1024, 768] f32, Wqkv [12, 768, 192] f32, bqkv [12, 192] f32,
  Wo [768, 768] f32, bo [768] f32
returns FULL output [8, 1024, 768] f32.

Strategy: pure data-parallel over batch (8 batches -> 8 NeuronCores), no
collectives.  Each core computes its batch end-to-end in bf16 matmuls.

Math notes:
  - softmax rows sum to 1 => attn @ (v + bv) = attn @ v + bv, and since the
    attention output is immediately projected, bv folds into the projection
    bias: bo2 = bo + concat(bv) @ Wo.  V-bias never touches the device.
  - x is transposed + bf16-cast on HOST (xT [768, 1024]) -- kills the 48 PE
    transposes, the f32 x DMA, and the startup serialization.
  - softmax is computed unnormalized; the denominator r[q] = sum_k et[k, q]
    is built by accumulating the bf16 exp chunks on DVE (S_h += et_chunk,
    flat contiguous [128,1024] ops) and a pair of CONCURRENT col-tiled
    ones-matmuls on the PE (~0.5us) that also broadcast r_a to psum
    partitions 0:64 and r_b to 64:128 (no DRAM bounce, no PE ones-column
    in the PV stationary).  This frees the PV stationary to hold exactly 64
    v-columns per head, so the two heads of a pair also run as CONCURRENT
    col-tiled matmuls (array cols 0-63 / 64-127).
  - output is written bf16 and upcast on host (halves the out DMA).

Schedule: flat software pipeline over 48 (pair, sk) chunks; pv(j-2) rides 2
chunk-slots behind scores(j); v-projection chunks fill pair 0; q/k
projections of pair t+1 injected mid-pair.  All PE instructions chained
with no-sync ordering edges so the Tile scheduler preserves the interleave.
"""

import math
import os

import numpy as np
import ml_dtypes

import concourse.bass as bass
import concourse.tile as tile
from concourse import bacc, mybir
from concourse.bass_utils import run_bass_kernel_spmd
from concourse.tile_rust import add_dep_helper

B, S, D, H, HD = 8, 1024, 768, 12, 64
SCALE = 1.0 / math.sqrt(D)
FP = mybir.dt.float32
BF = mybir.dt.bfloat16
KC = D // 128   # 6 contraction chunks
SC = S // 128   # 8 seq chunks
NQ = S // 512   # 2 free-dim chunks of 512
NP = H // 2     # 6 head pairs

AluOp = mybir.AluOpType
ActFn = mybir.ActivationFunctionType

# Results of the last hardware run (for test harness introspection).
last_results = None


def _build_kernel_body(tc, out_d, xT_d, wqk01_d, wqkr_d, wv_d, wo_d,
                       bqk_d, bo2_d):
    nc = tc.nc

    # Chain every TensorE instruction to the previous one with a no-sync
    # ordering edge: the Tile scheduler otherwise reorders the PE stream by
    # modeled readiness, undoing the deliberate scores/PV/QKV interleave.
    _pe_last = [None]

    def MM(*a, reuse_w=False, **k):
        inst = nc.tensor.matmul(*a, **k)
        if reuse_w:
            # stationary operand identical to the previous matmul in the
            # chained PE stream: skip the redundant LDWEIGHTS (bf16-safe)
            inst.ins.ldweights = False
        if _pe_last[0] is not None:
            add_dep_helper(inst.ins, _pe_last[0].ins, sync=False,
                           reason="pe-order")
        _pe_last[0] = inst
        return inst

    from contextlib import ExitStack

    with ExitStack() as ctx:
        wpool = ctx.enter_context(tc.tile_pool(name="weights", bufs=1))
        bigs = ctx.enter_context(tc.tile_pool(name="bigs", bufs=1))
        etp = ctx.enter_context(tc.tile_pool(name="et", bufs=2))
        spool = ctx.enter_context(tc.tile_pool(name="ssum", bufs=2))
        upool = ctx.enter_context(tc.tile_pool(name="usum", bufs=2))
        rpool = ctx.enter_context(tc.tile_pool(name="rbc", bufs=1))
        rcpool = ctx.enter_context(tc.tile_pool(name="rcp", bufs=1))
        outp = ctx.enter_context(tc.tile_pool(name="outstage", bufs=2))
        # psum budget (8 banks of [128 x 2KB]):
        #   scores transients 2 x 2 banks -- double-buffered so exp(j-1) never
        #     gates scores(j); outproj reuses these in the tail
        #   proj accumulator 1 x 2 banks -- qkv/v/r, so a 2.5us projection
        #     never holds a scores slot hostage (ScalarE starvation)
        #   pv accumulator 1 x 2 banks
        psq = ctx.enter_context(tc.tile_pool(name="ps_t", bufs=2, space="PSUM"))
        pspj = ctx.enter_context(tc.tile_pool(name="ps_pj", bufs=1, space="PSUM"))
        pspv = ctx.enter_context(tc.tile_pool(name="ps_pv", bufs=1, space="PSUM"))

        # ---- persistent sbuf tensors ----
        wqk01_sb = wpool.tile([128, KC, 256], BF)   # q/k blocks m=0 and m=6
        wqkr_sb = wpool.tile([128, KC, 2 * D - 256], BF)  # remaining m blocks
        wv_sb = wpool.tile([128, KC, D], BF)
        wo_sb = wpool.tile([128, KC, D], BF)
        bqk_sb = wpool.tile([128, 2 * KC], FP)
        bo_sb = wpool.tile([128, D], FP)
        oacc = wpool.tile([128, SC, D], BF)         # outproj kc0-4 partials
        xT = bigs.tile([128, KC, S], BF)
        ones_sb = wpool.tile([128, 128], BF)
        nc.vector.memset(ones_sb[:], 1.0)
        # qkT[:, m, :]: m 0..5 -> qT (heads 2m, 2m+1 on partitions 0:64,
        # 64:128), m 6..11 -> kT likewise.
        qkT = bigs.tile([128, 2 * KC, S], BF)
        vsb = bigs.tile([128, SC, D], BF)       # v in [s-part, sk, h*hd]
        outT = bigs.tile([128, KC, S], BF)

        # ---- input DMAs ----
        # All big tensors are prearranged on HOST into partition-contiguous
        # [128, n] layouts matching the sbuf tiles exactly: one descriptor
        # per partition (128 big descs instead of ~768 small) so the rings
        # drain fast.  wqk01 (q/k blocks of heads 0,1) rides first -- it
        # gates pair 0's scores.
        nc.scalar.dma_start(xT[:, 0:3, :], xT_d[:, 0:3 * S])
        nc.scalar.dma_start(xT[:, 3:KC, :], xT_d[:, 3 * S:])
        nc.sync.dma_start(wqk01_sb[:], wqk01_d[:, :])
        nc.sync.dma_start(bqk_sb[:], bqk_d.rearrange("(j p) -> p j", p=128))
        nc.sync.dma_start(wv_sb[:], wv_d[:, :])
        nc.sync.dma_start(wqkr_sb[:], wqkr_d[:, :])
        nc.sync.dma_start(wo_sb[:], wo_d[:, :])
        nc.sync.dma_start(
            bo_sb[:],
            bo2_d.rearrange("(a f) -> a f", a=1).partition_broadcast(128),
        )

        def wqk_block(m, kc):
            """stationary [128, 128] slice for q/k projection block m"""
            if m == 0:
                return wqk01_sb[:, kc, 0:128]
            if m == KC:
                return wqk01_sb[:, kc, 128:256]
            if m < KC:
                return wqkr_sb[:, kc, (m - 1) * 128:m * 128]
            return wqkr_sb[:, kc, 640 + (m - KC - 1) * 128:640 + (m - KC) * 128]

        def qkv_m(m):
            """project one 128-col block of q or k (m 0..5 q, 6..11 k)"""
            ps = pspj.tile([128, S], FP, tag="pj", name=f"qk_{m}")
            for kc in range(KC):
                lhsT = wqk_block(m, kc)
                for n in range(NQ):
                    MM(
                        ps[:, n * 512:(n + 1) * 512],
                        lhsT,
                        xT[:, kc, n * 512:(n + 1) * 512],
                        start=(kc == 0),
                        stop=(kc == KC - 1),
                        reuse_w=(n > 0),
                    )
            nc.vector.tensor_scalar_add(qkT[:, m, :], ps[:], bqk_sb[:, m:m + 1])

        def v_chunk(sc):
            ps = pspj.tile([128, S], FP, tag="pj", name=f"v_{sc}")
            for kc in range(KC):
                lhsT = xT[:, kc, sc * 128:(sc + 1) * 128]
                MM(ps[:, 0:512], lhsT, wv_sb[:, kc, 0:512],
                   start=(kc == 0), stop=(kc == KC - 1))
                MM(ps[:, 512:D], lhsT, wv_sb[:, kc, 512:D],
                   start=(kc == 0), stop=(kc == KC - 1), reuse_w=True)
            nc.vector.tensor_copy(vsb[:, sc, :], ps[:, 0:D])

        def scores_chunk(t, sk, et_t, s_t):
            for h01 in range(2):
                ps = psq.tile([128, S], FP, tag="ps", name=f"sc_{t}_{sk}_{h01}")
                lo, hi = h01 * 64, (h01 + 1) * 64
                lhsT = qkT[lo:hi, KC + t, sk * 128:(sk + 1) * 128]
                for n in range(NQ):
                    MM(
                        ps[:, n * 512:(n + 1) * 512],
                        lhsT,
                        qkT[lo:hi, t, n * 512:(n + 1) * 512],
                        start=True,
                        stop=True,
                        tile_position=(h01 * 64, 0),
                        reuse_w=(n > 0),
                    )
                nc.scalar.activation(
                    et_t[:, h01, sk, :], ps[:], ActFn.Exp, scale=SCALE
                )
                # running denominator: S_h += et chunk (flat contiguous
                # [128,1024] bf16 ops); first add at sk==1 consumes chunks
                # 0+1, skipping a separate init copy.  Head b's adds for
                # sk<=5 ride the otherwise-idle GpSimd so the DVE queue
                # stays shallow ahead of the pair-boundary u-copy.
                eng = nc.gpsimd if (h01 == 1 and sk <= 5) else nc.vector
                if sk == 1:
                    eng.tensor_tensor(s_t[h01][:], et_t[:, h01, 0, :],
                                      et_t[:, h01, 1, :], op=AluOp.add)
                elif sk > 1:
                    eng.tensor_tensor(s_t[h01][:], s_t[h01][:],
                                      et_t[:, h01, sk, :], op=AluOp.add)

        def pv_chunk(t, sk, et_t, pv_ps):
            # two heads as concurrent col-tiled matmuls: head a -> array
            # cols / psum partitions 0:64, head b -> 64:128 (tile_position
            # auto-derives from the psum slice base partition)
            for n in range(NQ):
                for h01 in range(2):
                    h = 2 * t + h01
                    MM(
                        pv_ps[h01 * 64:(h01 + 1) * 64, n * 512:(n + 1) * 512],
                        vsb[:, sk, h * 64:(h + 1) * 64],
                        et_t[:, h01, sk, n * 512:(n + 1) * 512],
                        start=(sk == 0),
                        stop=(sk == SC - 1),
                        skip_group_check=True,
                    )

        def pv_finalize(t, s_t, pv_ps):
            # Evacuate u from the PV psum IMMEDIATELY (single bf16 copy) so
            # the accumulator frees for the next pair (the strict in-order PE
            # queue would otherwise stall ~5us behind the next pair's first
            # PV matmul).  Then: r_a broadcast to psum partitions 0:64 and
            # r_b to 64:128 via two CONCURRENT col-tiled ones-matmuls
            # (partition-sum of S_h), copy to SBUF, fast Newton reciprocal,
            # and divide u*(1/r) into outT -- all off the PE critical path.
            u = upool.tile([128, S], BF, tag="u", name=f"u_{t}")
            nc.vector.tensor_copy(u[:], pv_ps[:])
            ps_r = pspj.tile([128, S], FP, tag="pj", name=f"r_{t}")
            for h01 in range(2):
                lo, hi = h01 * 64, (h01 + 1) * 64
                for n in range(NQ):
                    MM(ps_r[lo:hi, n * 512:(n + 1) * 512],
                       ones_sb[:, lo:hi],
                       s_t[h01][:, n * 512:(n + 1) * 512],
                       start=True, stop=True,
                       skip_group_check=True)
            rbc = rpool.tile([128, S], FP, tag="rbc", name=f"rbc_{t}")
            nc.vector.tensor_copy(rbc[:], ps_r[:])
            rcp = rcpool.tile([128, S], FP, tag="rcp", name=f"rcp_{t}")
            nc.vector.reciprocal_approx_fast(rcp[:], rbc[:])
            for h01 in range(2):
                lo, hi = h01 * 64, (h01 + 1) * 64
                # head b's divide is off the critical path for pairs 0-4
                # (outT only feeds the out-projection) -> idle GpSimd
                eng = nc.gpsimd if (h01 == 1 and t < NP - 1) else nc.vector
                eng.tensor_tensor(
                    outT[lo:hi, t, :],
                    u[lo:hi, :],
                    rcp[lo:hi, :],
                    op=AluOp.mult,
                )

        # ---- main pipeline ----
        # Flat software pipeline over 48 (pair, sk) chunks: pv(j-2) rides 2
        # chunk-slots behind scores(j), crossing pair boundaries, so neither
        # TensorE nor ScalarE ever drains.
        et_tiles = {}
        s_tiles = {}
        pv_tiles = {}

        def emit_pv(j):
            t, sk = j // SC, j % SC
            if sk == 0:
                pv_tiles[t] = pspv.tile([128, S], FP, tag="pv",
                                        name=f"pv_{t}")
            pv_chunk(t, sk, et_tiles[t], pv_tiles[t])
            if sk == SC - 1:
                pv_finalize(t, s_tiles[t], pv_tiles[t])
                del pv_tiles[t], et_tiles[t], s_tiles[t]

        def outproj_a(sc, pool, tag):
            """accumulate kc 0..4 of the output projection + bo into oacc"""
            ps = pool.tile([128, S], FP, tag=tag, name=f"oa_{sc}")
            for kc in range(KC - 1):
                lhsT = outT[:, kc, sc * 128:(sc + 1) * 128]
                MM(ps[:, 0:512], lhsT, wo_sb[:, kc, 0:512],
                   start=(kc == 0), stop=(kc == KC - 2))
                MM(ps[:, 512:D], lhsT, wo_sb[:, kc, 512:D],
                   start=(kc == 0), stop=(kc == KC - 2), reuse_w=True)
            nc.vector.tensor_tensor(oacc[:, sc, :], ps[:, 0:D], bo_sb[:],
                                    op=AluOp.add)

        # pair 0's q/k projections gate the whole pipeline
        qkv_m(0)
        qkv_m(KC)

        NCH = NP * SC
        for j in range(NCH):
            t, sk = j // SC, j % SC
            if sk == 0:
                et_tiles[t] = etp.tile([128, 2, SC, S], BF, tag="et",
                                       name=f"et_{t}")
                s_tiles[t] = [spool.tile([128, S], BF, tag=f"s{i}",
                                         name=f"s_{t}_{i}") for i in range(2)]
            scores_chunk(t, sk, et_tiles[t], s_tiles[t])
            if t == 0:
                v_chunk(sk)
            # q/k projections of the next pair ride at sk 2/3: the ~2.5us of
            # PE work right after pv(t,7) also absorbs the u-copy latency
            # that gates pv(t+1,0)'s psum slot.
            if t + 1 < NP:
                if sk == 2:
                    qkv_m(t + 1)
                elif sk == 3:
                    qkv_m(KC + t + 1)
            # out-projection kc0-4 partials overlap pair 5 (proj slot is
            # otherwise idle there: no more qkv, no more v); sc=0 waits
            # until sk=3 so finalize(4)'s mult has landed.
            if t == NP - 1 and sk >= 3:
                outproj_a(sk - 3, pspj, "pj")
            if j >= 2:
                emit_pv(j - 2)
        emit_pv(NCH - 2)
        emit_pv(NCH - 1)
        outproj_a(5, pspj, "pj")
        outproj_a(6, psq, "ps")
        outproj_a(7, psq, "ps")

        # ---- output projection: only the kc5 (pair 5) contraction remains ----
        for sc in range(SC):
            ps = psq.tile([128, S], FP, tag="ps", name=f"o_{sc}")
            lhsT = outT[:, KC - 1, sc * 128:(sc + 1) * 128]
            MM(ps[:, 0:512], lhsT, wo_sb[:, KC - 1, 0:512],
               start=True, stop=True)
            MM(ps[:, 512:D], lhsT, wo_sb[:, KC - 1, 512:D],
               start=True, stop=True, reuse_w=True)
            osb = outp.tile([128, D], BF, tag="osb", name=f"osb_{sc}")
            # (GPSIMD cannot access PSUM -- this add must stay on DVE)
            nc.vector.tensor_tensor(osb[:], ps[:, 0:D], oacc[:, sc, :],
                                    op=AluOp.add)
            (nc.scalar if sc % 2 else nc.sync).dma_start(
                out_d[sc * 128:(sc + 1) * 128, :], osb[:])


def build():
    """Build + compile the per-core Bass module. Returns the Bacc object.

    All big inputs are HOST-prearranged into partition-contiguous [128, n]
    layouts that mirror the sbuf tiles (row p = everything partition p
    holds, kc-major), so each DMA is 128 single-span descriptors.
    """
    nc = bacc.Bacc("TRN2", target_bir_lowering=False, debug=False, num_devices=B)
    xT_d = nc.dram_tensor("xT", [128, KC * S], BF, kind="ExternalInput").ap()
    wqk01_d = nc.dram_tensor("wqk01", [128, KC * 256], BF,
                             kind="ExternalInput").ap()
    wqkr_d = nc.dram_tensor("wqkr", [128, KC * (2 * D - 256)], BF,
                            kind="ExternalInput").ap()
    wv_d = nc.dram_tensor("wv", [128, KC * D], BF, kind="ExternalInput").ap()
    wo_d = nc.dram_tensor("wo", [128, KC * D], BF, kind="ExternalInput").ap()
    bqk_d = nc.dram_tensor("bqk", [2 * D], FP, kind="ExternalInput").ap()
    bo2_d = nc.dram_tensor("bo2", [D], FP, kind="ExternalInput").ap()
    out_d = nc.dram_tensor("out", [S, D], BF, kind="ExternalOutput").ap()
    with tile.TileContext(nc) as tc:
        _build_kernel_body(tc, out_d, xT_d, wqk01_d, wqkr_d, wv_d, wo_d,
                           bqk_d, bo2_d)
    nc.compile()
    return nc


def _prearrange(w):
    """[D, F] weight -> partition-contiguous [128, KC*F] (row p holds the
    kc-major sequence of rows kc*128+p), matching tile([128, KC, F])."""
    F = w.shape[1]
    return np.ascontiguousarray(
        w.reshape(KC, 128, F).transpose(1, 0, 2).reshape(128, KC * F))


def prep_weights(Wqkv, bqkv, Wo, bo):
    """Host-side weight packing (numpy only)."""
    # Wqkv [H, D, 3*HD] -> Wq_all/Wk_all/Wv_all [D, H*HD]
    Wq = np.transpose(Wqkv[:, :, 0:HD], (1, 0, 2)).reshape(D, D)
    Wk = np.transpose(Wqkv[:, :, HD:2 * HD], (1, 0, 2)).reshape(D, D)
    Wv = np.transpose(Wqkv[:, :, 2 * HD:], (1, 0, 2)).reshape(D, D)
    bq = bqkv[:, 0:HD].reshape(D)
    bk = bqkv[:, HD:2 * HD].reshape(D)
    bv = bqkv[:, 2 * HD:].reshape(D)
    bqk = np.concatenate([bq, bk])  # [2D]
    bo2 = bo.astype(np.float64) + bv.astype(np.float64) @ Wo.astype(np.float64)
    bf16 = ml_dtypes.bfloat16
    wqk01 = np.concatenate([Wq[:, 0:128], Wk[:, 0:128]], axis=1)  # [D, 256]
    wqkr = np.concatenate([Wq[:, 128:D], Wk[:, 128:D]], axis=1)  # [D, 1280]
    return {
        "wqk01": _prearrange(wqk01.astype(bf16)),
        "wqkr": _prearrange(wqkr.astype(bf16)),
        "wv": _prearrange(Wv.astype(bf16)),
        "wo": _prearrange(Wo.astype(bf16)),
        "bqk": np.ascontiguousarray(bqk.astype(np.float32)),
        "bo2": np.ascontiguousarray(bo2.astype(np.float32)),
    }


def prep_core_inputs(x, Wqkv, bqkv, Wo, bo):
    """Full host-side preprocessing -> list of per-core input maps."""
    w = prep_weights(np.asarray(Wqkv), np.asarray(bqkv), np.asarray(Wo),
                     np.asarray(bo))
    x = np.asarray(x, dtype=np.float32)
    bf16 = ml_dtypes.bfloat16
    return [
        {"xT": _prearrange(np.ascontiguousarray(x[i].T).astype(bf16)), **w}
        for i in range(B)
    ]


_nc_cache = None


def kernel(x, Wqkv, bqkv, Wo, bo):
    global _nc_cache, last_results
    if _nc_cache is None:
        _nc_cache = build()
    nc = _nc_cache
    in_maps = prep_core_inputs(x, Wqkv, bqkv, Wo, bo)
    res = run_bass_kernel_spmd(
        nc, in_maps, core_ids=list(range(B)),
        trace=bool(os.environ.get("KERNEL_TRACE")),
    )
    last_results = res
    out = np.stack([res.results[i]["out"] for i in range(B)], axis=0)
    return out.astype(np.float32)


# revision 50
# speedup vs baseline: 1.0323x; 1.0237x over previous
"""Trainium2 Bass kernel for nn_Attention: per-head QKV attention + out-proj.

Contract: kernel(**inputs) takes FULL unsharded inputs
  x [8, 1024, 768] f32, Wqkv [12, 768, 192] f32, bqkv [12, 192] f32,
  Wo [768, 768] f32, bo [768] f32
returns FULL output [8, 1024, 768] f32.

Strategy: pure data-parallel over batch (8 batches -> 8 NeuronCores), no
collectives.  Each core computes its batch end-to-end in bf16 matmuls.

Math notes:
  - softmax rows sum to 1 => attn @ (v + bv) = attn @ v + bv, and since the
    attention output is immediately projected, bv folds into the projection
    bias: bo2 = bo + concat(bv) @ Wo.  V-bias never touches the device.
  - x is transposed + bf16-cast on HOST (xT [768, 1024]) -- kills the 48 PE
    transposes, the f32 x DMA, and the startup serialization.
  - softmax is computed unnormalized; the denominator r[q] = sum_k et[k, q]
    is built by accumulating the bf16 exp chunks on DVE (S_h += et_chunk,
    flat contiguous [128,1024] ops) and a pair of CONCURRENT col-tiled
    ones-matmuls on the PE (~0.5us) that also broadcast r_a to psum
    partitions 0:64 and r_b to 64:128 (no DRAM bounce, no PE ones-column
    in the PV stationary).  This frees the PV stationary to hold exactly 64
    v-columns per head, so the two heads of a pair also run as CONCURRENT
    col-tiled matmuls (array cols 0-63 / 64-127).
  - output is written bf16 and upcast on host (halves the out DMA).

Schedule: flat software pipeline over 48 (pair, sk) chunks; pv(j-2) rides 2
chunk-slots behind scores(j); v-projection chunks fill pair 0; q/k
projections of pair t+1 injected mid-pair.  All PE instructions chained
with no-sync ordering edges so the Tile scheduler preserves the interleave.
"""

import math
import os

import numpy as np
import ml_dtypes

import concourse.bass as bass
import concourse.tile as tile
from concourse import bacc, mybir
from concourse.bass_utils import run_bass_kernel_spmd
from concourse.tile_rust import add_dep_helper

B, S, D, H, HD = 8, 1024, 768, 12, 64
SCALE = 1.0 / math.sqrt(D)
FP = mybir.dt.float32
BF = mybir.dt.bfloat16
KC = D // 128   # 6 contraction chunks
SC = S // 128   # 8 seq chunks
NQ = S // 512   # 2 free-dim chunks of 512
NP = H // 2     # 6 head pairs

AluOp = mybir.AluOpType
ActFn = mybir.ActivationFunctionType

# Results of the last hardware run (for test harness introspection).
last_results = None


def _build_kernel_body(tc, out_d, xT_d, wqk01_d, wqkr_d, wv_d, wo_d,
                       bqk_d, bo2_d):
    nc = tc.nc

    # Chain every TensorE instruction to the previous one with a no-sync
    # ordering edge: the Tile scheduler otherwise reorders the PE stream by
    # modeled readiness, undoing the deliberate scores/PV/QKV interleave.
    _pe_last = [None]

    def MM(*a, reuse_w=False, **k):
        inst = nc.tensor.matmul(*a, **k)
        if reuse_w:
            # stationary operand identical to the previous matmul in the
            # chained PE stream: skip the redundant LDWEIGHTS (bf16-safe)
            inst.ins.ldweights = False
        if _pe_last[0] is not None:
            add_dep_helper(inst.ins, _pe_last[0].ins, sync=False,
                           reason="pe-order")
        _pe_last[0] = inst
        return inst

    from contextlib import ExitStack

    with ExitStack() as ctx:
        wpool = ctx.enter_context(tc.tile_pool(name="weights", bufs=1))
        bigs = ctx.enter_context(tc.tile_pool(name="bigs", bufs=1))
        etp = ctx.enter_context(tc.tile_pool(name="et", bufs=2))
        spool = ctx.enter_context(tc.tile_pool(name="ssum", bufs=2))
        upool = ctx.enter_context(tc.tile_pool(name="usum", bufs=2))
        rpool = ctx.enter_context(tc.tile_pool(name="rbc", bufs=1))
        rcpool = ctx.enter_context(tc.tile_pool(name="rcp", bufs=1))
        outp = ctx.enter_context(tc.tile_pool(name="outstage", bufs=2))
        # psum budget (8 banks of [128 x 2KB]):
        #   scores transients 2 x 2 banks -- double-buffered so exp(j-1) never
        #     gates scores(j); outproj reuses these in the tail
        #   proj accumulator 1 x 2 banks -- qkv/v/r, so a 2.5us projection
        #     never holds a scores slot hostage (ScalarE starvation)
        #   pv accumulator 1 x 2 banks
        psq = ctx.enter_context(tc.tile_pool(name="ps_t", bufs=2, space="PSUM"))
        pspj = ctx.enter_context(tc.tile_pool(name="ps_pj", bufs=1, space="PSUM"))
        pspv = ctx.enter_context(tc.tile_pool(name="ps_pv", bufs=1, space="PSUM"))

        # ---- persistent sbuf tensors ----
        wqk01_sb = wpool.tile([128, KC, 256], BF)   # q/k blocks m=0 and m=6
        wqkr_sb = wpool.tile([128, KC, 2 * D - 256], BF)  # remaining m blocks
        wv_sb = wpool.tile([128, KC, D], BF)
        wo_sb = wpool.tile([128, KC, D], BF)
        bqk_sb = wpool.tile([128, 2 * KC], FP)
        bo_sb = wpool.tile([128, D], FP)
        oacc = wpool.tile([128, SC, D], BF)         # outproj kc0-4 partials
        xT = bigs.tile([128, KC, S], BF)
        ones_sb = wpool.tile([128, 128], BF)
        nc.vector.memset(ones_sb[:], 1.0)
        # qkT[:, m, :]: m 0..5 -> qT (heads 2m, 2m+1 on partitions 0:64,
        # 64:128), m 6..11 -> kT likewise.
        qkT = bigs.tile([128, 2 * KC, S], BF)
        vsb = bigs.tile([128, SC, D], BF)       # v in [s-part, sk, h*hd]
        outT = bigs.tile([128, KC, S], BF)

        # ---- input DMAs ----
        # All big tensors are prearranged on HOST into partition-contiguous
        # [128, n] layouts matching the sbuf tiles exactly: one descriptor
        # per partition (128 big descs instead of ~768 small) so the rings
        # drain fast.  wqk01 (q/k blocks of heads 0,1) rides first -- it
        # gates pair 0's scores.
        nc.scalar.dma_start(xT[:, 0:3, :], xT_d[:, 0:3 * S])
        nc.scalar.dma_start(xT[:, 3:KC, :], xT_d[:, 3 * S:])
        nc.sync.dma_start(wqk01_sb[:], wqk01_d[:, :])
        nc.sync.dma_start(bqk_sb[:], bqk_d.rearrange("(j p) -> p j", p=128))
        nc.sync.dma_start(wv_sb[:], wv_d[:, :])
        nc.sync.dma_start(wqkr_sb[:], wqkr_d[:, :])
        nc.sync.dma_start(wo_sb[:], wo_d[:, :])
        nc.sync.dma_start(
            bo_sb[:],
            bo2_d.rearrange("(a f) -> a f", a=1).partition_broadcast(128),
        )

        def wqk_block(m, kc):
            """stationary [128, 128] slice for q/k projection block m"""
            if m == 0:
                return wqk01_sb[:, kc, 0:128]
            if m == KC:
                return wqk01_sb[:, kc, 128:256]
            if m < KC:
                return wqkr_sb[:, kc, (m - 1) * 128:m * 128]
            return wqkr_sb[:, kc, 640 + (m - KC - 1) * 128:640 + (m - KC) * 128]

        def qkv_m(m):
            """project one 128-col block of q or k (m 0..5 q, 6..11 k)"""
            ps = pspj.tile([128, S], FP, tag="pj", name=f"qk_{m}")
            for kc in range(KC):
                lhsT = wqk_block(m, kc)
                for n in range(NQ):
                    MM(
                        ps[:, n * 512:(n + 1) * 512],
                        lhsT,
                        xT[:, kc, n * 512:(n + 1) * 512],
                        start=(kc == 0),
                        stop=(kc == KC - 1),
                        reuse_w=(n > 0),
                    )
            nc.vector.tensor_scalar_add(qkT[:, m, :], ps[:], bqk_sb[:, m:m + 1])

        def v_chunk(sc):
            ps = pspj.tile([128, S], FP, tag="pj", name=f"v_{sc}")
            for kc in range(KC):
                lhsT = xT[:, kc, sc * 128:(sc + 1) * 128]
                MM(ps[:, 0:512], lhsT, wv_sb[:, kc, 0:512],
                   start=(kc == 0), stop=(kc == KC - 1))
                MM(ps[:, 512:D], lhsT, wv_sb[:, kc, 512:D],
                   start=(kc == 0), stop=(kc == KC - 1), reuse_w=True)
            nc.vector.tensor_copy(vsb[:, sc, :], ps[:, 0:D])

        def scores_chunk(t, sk, et_t, s_t):
            for h01 in range(2):
                ps = psq.tile([128, S], FP, tag="ps", name=f"sc_{t}_{sk}_{h01}")
                lo, hi = h01 * 64, (h01 + 1) * 64
                lhsT = qkT[lo:hi, KC + t, sk * 128:(sk + 1) * 128]
                for n in range(NQ):
                    MM(
                        ps[:, n * 512:(n + 1) * 512],
                        lhsT,
                        qkT[lo:hi, t, n * 512:(n + 1) * 512],
                        start=True,
                        stop=True,
                        tile_position=(h01 * 64, 0),
                        reuse_w=(n > 0),
                    )
                nc.scalar.activation(
                    et_t[:, h01, sk, :], ps[:], ActFn.Exp, scale=SCALE
                )
                # running denominator: S_h += et chunk (flat contiguous
                # [128,1024] bf16 ops); first add at sk==1 consumes chunks
                # 0+1, skipping a separate init copy.  Head b's adds for
                # sk<=5 ride the otherwise-idle GpSimd so the DVE queue
                # stays shallow ahead of the pair-boundary u-copy.
                eng = nc.gpsimd if (h01 == 1 and sk <= 5) else nc.vector
                if sk == 1:
                    eng.tensor_tensor(s_t[h01][:], et_t[:, h01, 0, :],
                                      et_t[:, h01, 1, :], op=AluOp.add)
                elif sk > 1:
                    eng.tensor_tensor(s_t[h01][:], s_t[h01][:],
                                      et_t[:, h01, sk, :], op=AluOp.add)

        def pv_chunk(t, sk, et_t, pv_ps):
            # two heads as concurrent col-tiled matmuls: head a -> array
            # cols / psum partitions 0:64, head b -> 64:128 (tile_position
            # auto-derives from the psum slice base partition)
            for n in range(NQ):
                for h01 in range(2):
                    h = 2 * t + h01
                    MM(
                        pv_ps[h01 * 64:(h01 + 1) * 64, n * 512:(n + 1) * 512],
                        vsb[:, sk, h * 64:(h + 1) * 64],
                        et_t[:, h01, sk, n * 512:(n + 1) * 512],
                        start=(sk == 0),
                        stop=(sk == SC - 1),
                        skip_group_check=True,
                    )

        def pv_finalize(t, s_t, pv_ps):
            # Evacuate u from the PV psum IMMEDIATELY (single bf16 copy) so
            # the accumulator frees for the next pair (the strict in-order PE
            # queue would otherwise stall ~5us behind the next pair's first
            # PV matmul).  Then: r_a broadcast to psum partitions 0:64 and
            # r_b to 64:128 via two CONCURRENT col-tiled ones-matmuls
            # (partition-sum of S_h), copy to SBUF, fast Newton reciprocal,
            # and divide u*(1/r) into outT -- all off the PE critical path.
            u = upool.tile([128, S], BF, tag="u", name=f"u_{t}")
            nc.vector.tensor_copy(u[:], pv_ps[:])
            ps_r = pspj.tile([128, S], FP, tag="pj", name=f"r_{t}")
            for h01 in range(2):
                lo, hi = h01 * 64, (h01 + 1) * 64
                for n in range(NQ):
                    MM(ps_r[lo:hi, n * 512:(n + 1) * 512],
                       ones_sb[:, lo:hi],
                       s_t[h01][:, n * 512:(n + 1) * 512],
                       start=True, stop=True,
                       skip_group_check=True)
            rbc = rpool.tile([128, S], FP, tag="rbc", name=f"rbc_{t}")
            nc.vector.tensor_copy(rbc[:], ps_r[:])
            rcp = rcpool.tile([128, S], FP, tag="rcp", name=f"rcp_{t}")
            nc.vector.reciprocal_approx_fast(rcp[:], rbc[:])
            for h01 in range(2):
                lo, hi = h01 * 64, (h01 + 1) * 64
                # head b's divide is off the critical path for pairs 0-4
                # (outT only feeds the out-projection) -> idle GpSimd
                eng = nc.gpsimd if (h01 == 1 and t < NP - 1) else nc.vector
                eng.tensor_tensor(
                    outT[lo:hi, t, :],
                    u[lo:hi, :],
                    rcp[lo:hi, :],
                    op=AluOp.mult,
                )

        # ---- main pipeline ----
        # Flat software pipeline over 48 (pair, sk) chunks: pv(j-2) rides 2
        # chunk-slots behind scores(j), crossing pair boundaries, so neither
        # TensorE nor ScalarE ever drains.
        et_tiles = {}
        s_tiles = {}
        pv_tiles = {}

        def emit_pv(j):
            t, sk = j // SC, j % SC
            if sk == 0:
                pv_tiles[t] = pspv.tile([128, S], FP, tag="pv",
                                        name=f"pv_{t}")
            pv_chunk(t, sk, et_tiles[t], pv_tiles[t])
            if sk == SC - 1:
                pv_finalize(t, s_tiles[t], pv_tiles[t])
                del pv_tiles[t], et_tiles[t], s_tiles[t]

        def outproj_a(sc, pool, tag):
            """accumulate kc 0..4 of the output projection + bo into oacc"""
            ps = pool.tile([128, S], FP, tag=tag, name=f"oa_{sc}")
            for kc in range(KC - 1):
                lhsT = outT[:, kc, sc * 128:(sc + 1) * 128]
                MM(ps[:, 0:512], lhsT, wo_sb[:, kc, 0:512],
                   start=(kc == 0), stop=(kc == KC - 2))
                MM(ps[:, 512:D], lhsT, wo_sb[:, kc, 512:D],
                   start=(kc == 0), stop=(kc == KC - 2), reuse_w=True)
            nc.vector.tensor_tensor(oacc[:, sc, :], ps[:, 0:D], bo_sb[:],
                                    op=AluOp.add)

        # pair 0's q/k projections gate the whole pipeline
        qkv_m(0)
        qkv_m(KC)

        NCH = NP * SC
        for j in range(NCH):
            t, sk = j // SC, j % SC
            if sk == 0:
                et_tiles[t] = etp.tile([128, 2, SC, S], BF, tag="et",
                                       name=f"et_{t}")
                s_tiles[t] = [spool.tile([128, S], BF, tag=f"s{i}",
                                         name=f"s_{t}_{i}") for i in range(2)]
            scores_chunk(t, sk, et_tiles[t], s_tiles[t])
            if t == 0:
                v_chunk(sk)
            # q/k projections of the next pair ride at sk 2/3: the ~2.5us of
            # PE work right after pv(t,7) also absorbs the u-copy latency
            # that gates pv(t+1,0)'s psum slot.
            if t + 1 < NP:
                if sk == 2:
                    qkv_m(t + 1)
                elif sk == 3:
                    qkv_m(KC + t + 1)
            # out-projection kc0-4 partials overlap pair 5 (proj slot is
            # otherwise idle there: no more qkv, no more v); sc=0 waits
            # until sk=4 -- finalize(4)'s divide drains through the GpSimd
            # queue ~2 chunks after issue, and at sk=3 the PE stalls ~3.5us
            # on it (measured).
            if t == NP - 1 and sk >= 4:
                outproj_a(sk - 4, pspj, "pj")
            if j >= 2:
                emit_pv(j - 2)
        emit_pv(NCH - 2)
        emit_pv(NCH - 1)
        # psq-first ordering: the psq slots free as soon as the last exps
        # read them, while the pj slot waits for finalize(5)'s rbc copy
        outproj_a(4, psq, "ps")
        outproj_a(6, psq, "ps")
        outproj_a(5, pspj, "pj")
        outproj_a(7, psq, "ps")

        # ---- output projection: only the kc5 (pair 5) contraction remains ----
        for sc in range(SC):
            ps = psq.tile([128, S], FP, tag="ps", name=f"o_{sc}")
            lhsT = outT[:, KC - 1, sc * 128:(sc + 1) * 128]
            MM(ps[:, 0:512], lhsT, wo_sb[:, KC - 1, 0:512],
               start=True, stop=True)
            MM(ps[:, 512:D], lhsT, wo_sb[:, KC - 1, 512:D],
               start=True, stop=True, reuse_w=True)
            osb = outp.tile([128, D], BF, tag="osb", name=f"osb_{sc}")
            # (GPSIMD cannot access PSUM -- this add must stay on DVE)
            nc.vector.tensor_tensor(osb[:], ps[:, 0:D], oacc[:, sc, :],
                                    op=AluOp.add)
            (nc.scalar if sc % 2 else nc.sync).dma_start(
                out_d[sc * 128:(sc + 1) * 128, :], osb[:])


def build():
    """Build + compile the per-core Bass module. Returns the Bacc object.

    All big inputs are HOST-prearranged into partition-contiguous [128, n]
    layouts that mirror the sbuf tiles (row p = everything partition p
    holds, kc-major), so each DMA is 128 single-span descriptors.
    """
    nc = bacc.Bacc("TRN2", target_bir_lowering=False, debug=False, num_devices=B)
    xT_d = nc.dram_tensor("xT", [128, KC * S], BF, kind="ExternalInput").ap()
    wqk01_d = nc.dram_tensor("wqk01", [128, KC * 256], BF,
                             kind="ExternalInput").ap()
    wqkr_d = nc.dram_tensor("wqkr", [128, KC * (2 * D - 256)], BF,
                            kind="ExternalInput").ap()
    wv_d = nc.dram_tensor("wv", [128, KC * D], BF, kind="ExternalInput").ap()
    wo_d = nc.dram_tensor("wo", [128, KC * D], BF, kind="ExternalInput").ap()
    bqk_d = nc.dram_tensor("bqk", [2 * D], FP, kind="ExternalInput").ap()
    bo2_d = nc.dram_tensor("bo2", [D], FP, kind="ExternalInput").ap()
    out_d = nc.dram_tensor("out", [S, D], BF, kind="ExternalOutput").ap()
    with tile.TileContext(nc) as tc:
        _build_kernel_body(tc, out_d, xT_d, wqk01_d, wqkr_d, wv_d, wo_d,
                           bqk_d, bo2_d)
    nc.compile()
    return nc


def _prearrange(w):
    """[D, F] weight -> partition-contiguous [128, KC*F] (row p holds the
    kc-major sequence of rows kc*128+p), matching tile([128, KC, F])."""
    F = w.shape[1]
    return np.ascontiguousarray(
        w.reshape(KC, 128, F).transpose(1, 0, 2).reshape(128, KC * F))


def prep_weights(Wqkv, bqkv, Wo, bo):
    """Host-side weight packing (numpy only)."""
    # Wqkv [H, D, 3*HD] -> Wq_all/Wk_all/Wv_all [D, H*HD]
    Wq = np.transpose(Wqkv[:, :, 0:HD], (1, 0, 2)).reshape(D, D)
    Wk = np.transpose(Wqkv[:, :, HD:2 * HD], (1, 0, 2)).reshape(D, D)
    Wv = np.transpose(Wqkv[:, :, 2 * HD:], (1, 0, 2)).reshape(D, D)
    bq = bqkv[:, 0:HD].reshape(D)
    bk = bqkv[:, HD:2 * HD].reshape(D)
    bv = bqkv[:, 2 * HD:].reshape(D)
    bqk = np.concatenate([bq, bk])  # [2D]
    bo2 = bo.astype(np.float64) + bv.astype(np.float64) @ Wo.astype(np.float64)
    bf16 = ml_dtypes.bfloat16
    wqk01 = np.concatenate([Wq[:, 0:128], Wk[:, 0:128]], axis=1)  # [D, 256]
    wqkr = np.concatenate([Wq[:, 128:D], Wk[:, 128:D]], axis=1)  # [D, 1280]
    return {
        "wqk01": _prearrange(wqk01.astype(bf16)),
        "wqkr": _prearrange(wqkr.astype(bf16)),
        "wv": _prearrange(Wv.astype(bf16)),
        "wo": _prearrange(Wo.astype(bf16)),
        "bqk": np.ascontiguousarray(bqk.astype(np.float32)),
        "bo2": np.ascontiguousarray(bo2.astype(np.float32)),
    }


def prep_core_inputs(x, Wqkv, bqkv, Wo, bo):
    """Full host-side preprocessing -> list of per-core input maps."""
    w = prep_weights(np.asarray(Wqkv), np.asarray(bqkv), np.asarray(Wo),
                     np.asarray(bo))
    x = np.asarray(x, dtype=np.float32)
    bf16 = ml_dtypes.bfloat16
    return [
        {"xT": _prearrange(np.ascontiguousarray(x[i].T).astype(bf16)), **w}
        for i in range(B)
    ]


_nc_cache = None


def kernel(x, Wqkv, bqkv, Wo, bo):
    global _nc_cache, last_results
    if _nc_cache is None:
        _nc_cache = build()
    nc = _nc_cache
    in_maps = prep_core_inputs(x, Wqkv, bqkv, Wo, bo)
    res = run_bass_kernel_spmd(
        nc, in_maps, core_ids=list(range(B)),
        trace=bool(os.environ.get("KERNEL_TRACE")),
    )
    last_results = res
    out = np.stack([res.results[i]["out"] for i in range(B)], axis=0)
    return out.astype(np.float32)
